# revision 13
# baseline (speedup 1.0000x reference)
"""Trainium2 Bass kernel for nn_MultiHeadAttention_61546881352366.

The reference module's observable output is NOT attention: the attention
result is dead code in the original torch module.  The output is

    out = fc0(concat_h(v @ Wv_h^T)) = (v @ Wcat^T) @ W0^T + b0

with Wcat = Wv.reshape(H*D, C).  Two chained linear maps fuse into one:

    out = v @ (W0 @ Wcat)^T + b0 = v @ WcT + b0,   WcT = (W0 @ Wcat)^T

so the device work is a single [B*T, C] @ [C, C] matmul plus a bias add.
k and q are unused.

Sharding: data-parallel over batch (B == 8 == n_cores); each core computes
one batch element's [2048, 1024] @ [1024, 1024] product in bf16 (rel err
~3e-3 vs the 2e-2 gate).

Per-core kernel structure (measured-HW driven):
- w is the STATIONARY matmul operand (lhsT = 128x128 block of WcT), v the
  moving operand (512-row t-segments).  With the weight-load dedupe below,
  each loaded weight block is reused across consecutive matmuls.
- Phase A (runs during the DMA fill): the first 512-token segment for all
  8 output-channel blocks, j-outer/k-inner -- consumes only w blocks (as
  they stream in) plus the first v segment, so the PE starts ~3.5us in and
  never waits for the rest of the 6.5MB fill.
- Main phase: j-outer / k-mid / t-inner over the remaining 3 segments;
  each ldweights is shared by 3 matmuls.
- Tail: the last block runs t-mid/k-inner with per-segment output DMAs
  (final half-segment separate) so only a 64KB DMA trails the last matmul.
- Drain: PSUM -> SBUF bf16 with the bias added via per-partition
  tensor_scalar on the vector engine; output written transposed
  (outT[c, t]) with 2-4KB contiguous DMA lines; host transposes+upcasts.
- PE warmup matmuls ramp the HAM clock gate (1.2 -> 2.4 GHz) during fill;
  sized to end right as the first v half lands (~2.5us) -- the cold-busy
  budget before the clock flip is constant, so longer dummy warmup only
  delays real work and shorter warmup risks an idle gap delaying the flip.
- dedupe_ldweights: the Tile legalizer emits one InstLdweights per matmul
  even for identical consecutive stationary operands; redundant wait-free
  loads are deleted post-schedule (the PE array retains weights), removing
  ~105 x 128-cycle reloads from the PE critical path.

`reps` repeats the body inside one NEFF (timing harness only; graded path
uses reps=1).
"""

import numpy as np

import concourse.bacc as bacc
import concourse.mybir as mybir
from concourse.tile import TileContext
from concourse.bass_utils import run_bass_kernel_spmd

B, T, C = 8, 2048, 1024
H, D = 16, 64
P = 128
KT = C // P      # 8 contraction tiles
JT = C // P      # 8 output-channel blocks
NT = 4           # t segments
TS = T // NT     # 512 rows per segment (one PSUM bank)

_FP32 = mybir.dt.float32
_BF16 = mybir.dt.bfloat16

N_WARMUP = 6


def _ldw_sig(ap):
    return (ap.memref, ap.offset, str(ap.ap), str(ap.dtype))


def dedupe_ldweights(nc):
    """Delete redundant InstLdweights (identical AP as the previously loaded
    one, only InstMatmult in between on the PE stream, no waits/updates).
    The PE array retains loaded weights across matmuls, and these loads carry
    no semaphore updates, so removal is semantically transparent."""
    n_del = 0
    for blk in nc.main_func.blocks:
        insts = blk.instructions
        last_sig = None
        keep = []
        for inst in insts:
            tn = type(inst).__name__
            if tn == "InstLdweights":
                si = inst.sync_info
                clean = not si or (len(si.on_wait) == 0 and len(si.on_update) == 0)
                sig = _ldw_sig(inst.ins[0])
                if clean and sig == last_sig:
                    n_del += 1
                    continue
                last_sig = sig
                keep.append(inst)
            elif tn == "InstMatmult":
                keep.append(inst)
            else:
                if getattr(inst, "engine", None) == mybir.EngineType.PE:
                    last_sig = None
                keep.append(inst)
        if n_del:
            del insts[:]
            insts.extend(keep)
    return n_del


def _build(reps=1):
    nc = bacc.Bacc()
    # all v strips in SBUF-order layout: vR[p, ((s*KT + k)*TS + t)] so every
    # strip DMA is contiguous on both sides (128 large descriptors instead
    # of ~1024 fragmented ones -- cuts DGE/descriptor latency).  Strip 0
    # ships as two k-halves (a k-slice of the [P, KT, TS] tile is contiguous
    # per partition), and the first 512KB half already enables the k0-3
    # matmuls of ALL output blocks.
    vR = nc.dram_tensor("vR", [P, NT * KT * TS], _BF16, kind="ExternalInput")
    # w chunks (j0 | j1-3 | j4-7) in SBUF-order layout: per partition
    # [k, j-within-chunk] contiguous, so each chunk DMA is contiguous on
    # both sides (128 descriptors instead of ~1024 fragmented 256B ones).
    wR0 = nc.dram_tensor("wR0", [P, KT * P], _BF16, kind="ExternalInput")
    wR1 = nc.dram_tensor("wR1", [P, KT * 3 * P], _BF16, kind="ExternalInput")
    wR2 = nc.dram_tensor("wR2", [P, KT * 4 * P], _BF16, kind="ExternalInput")
    bT = nc.dram_tensor("bT", [P, JT], _FP32, kind="ExternalInput")
    outT = nc.dram_tensor("outT", [C, T], _BF16, kind="ExternalOutput")

    vR_r = vR[:, :].rearrange("p (s k t) -> p s k t", s=NT, k=KT)  # [128,4,KT,TS]
    wR_r = (
        wR0[:, :].rearrange("p (k j) -> p k j", k=KT),
        wR1[:, :].rearrange("p (k j) -> p k j", k=KT),
        wR2[:, :].rearrange("p (k j) -> p k j", k=KT),
    )

    with TileContext(nc) as tc:
        with (
            tc.tile_pool(name="wpool", bufs=1) as wpool,
            tc.tile_pool(name="vpool", bufs=1) as vpool,
            tc.tile_pool(name="bpool", bufs=1) as bpool,
            tc.tile_pool(name="opool", bufs=9) as opool,
            tc.tile_pool(name="pspool", bufs=8, space="PSUM") as pspool,
        ):
            scratch = bpool.tile([P, TS], _BF16, name="scratch", tag="scratch")
            nc.vector.memset(scratch, 0.0)
            ps_w = pspool.tile([P, TS], _FP32, name="ps_w", tag="ps")
            for _ in range(N_WARMUP):
                nc.tensor.matmul(
                    ps_w, lhsT=scratch[:, :P], rhs=scratch, start=True, stop=True
                )

            if reps == 1:
                _one_pass(nc, tc, vR_r, wR_r, bT, outT, wpool, vpool, bpool, opool, pspool)
            else:
                with tc.For_i(0, reps, 1, hint_engines=(mybir.EngineType.PE,)):
                    _one_pass(nc, tc, vR_r, wR_r, bT, outT, wpool, vpool, bpool, opool, pspool)
    dedupe_ldweights(nc)
    nc.compile()
    return nc


def _one_pass(nc, tc, vR_r, wR_r, bT, outT, wpool, vpool, bpool, opool, pspool):
    w_sb = [None] * JT
    v_sb = [None] * NT
    ob_sb = [None] * JT

    def dma_v(t):
        v_t = vpool.tile([P, KT, TS], _BF16, name=f"v_{t}", tag=f"v_{t}")
        nc.scalar.dma_start(out=v_t, in_=vR_r[:, t])
        v_sb[t] = v_t

    # DMA order: w front-loaded (phase A consumes w_j every ~1.7us), then
    # the remaining v strips, first needed when the main phase starts.
    # w ships as 3 chunks (w0 | w1-3 | w4-7) from one SBUF tile: HWDGE
    # descriptor generation serializes per ring (~0.6us/DMA), so fewer,
    # larger transfers pull the later v strips' arrival earlier.
    b_sb = bpool.tile([P, JT], _FP32, name="b_sb", tag="b_sb", bufs=2)
    nc.sync.dma_start(out=b_sb, in_=bT[:, :])
    w_c0 = wpool.tile([P, KT, P], _BF16, name="w_c0", tag="w_c0")
    w_c1 = wpool.tile([P, KT, 3 * P], _BF16, name="w_c1", tag="w_c1")
    w_c2 = wpool.tile([P, KT, 4 * P], _BF16, name="w_c2", tag="w_c2")
    w_sb[0] = w_c0[:, :, :]
    for j in range(1, 4):
        w_sb[j] = w_c1[:, :, (j - 1) * P : j * P]
    for j in range(4, JT):
        w_sb[j] = w_c2[:, :, (j - 4) * P : (j - 3) * P]
    # w0 rides the sync ring in parallel with v0's first half on the scalar
    # ring, so the first matmul's inputs land as early as possible.
    nc.sync.dma_start(out=w_c0, in_=wR_r[0])
    v0 = vpool.tile([P, KT, TS], _BF16, name="v_0", tag="v_0")
    # scalar-ring order matches first-need times: v0's first k-half (~2.5us),
    # w_c1 (j1 at ~4.3us), w_c2 (j4 at ~6.8us), v0's second k-half (pass 2
    # at ~9us), then the main-phase strips.
    nc.scalar.dma_start(out=v0[:, 0 : KT // 2, :], in_=vR_r[:, 0, 0 : KT // 2, :])
    nc.scalar.dma_start(out=w_c1, in_=wR_r[1])
    nc.scalar.dma_start(out=w_c2, in_=wR_r[2])
    nc.scalar.dma_start(out=v0[:, KT // 2 : KT, :], in_=vR_r[:, 0, KT // 2 : KT, :])
    v_sb[0] = v0
    dma_v(1)
    dma_v(2)
    dma_v(3)

    def drain(j, t, ps_t):
        nc.vector.tensor_scalar(
            out=ob_sb[j][:, t * TS : (t + 1) * TS],
            in0=ps_t,
            scalar1=b_sb[:, j : j + 1],
            scalar2=None,
            op0=mybir.AluOpType.add,
        )

    # --- Phase A: t0 for all j, in two k-half passes.  The first 512KB
    # k-half of v0 enables the k0-3 matmuls of ALL output blocks, so the
    # PE has ~7us of work the moment the first contiguous half lands;
    # accumulation groups stay open across the pass boundary. ------------
    psA = [None] * JT
    for kh in range(2):
        for j in range(JT):
            if kh == 0:
                ob_sb[j] = opool.tile([P, T], _BF16, name=f"ob_{j}", tag="ob")
                psA[j] = pspool.tile([P, TS], _FP32, name=f"psA_{j}", tag="ps")
            for k in range(kh * (KT // 2), (kh + 1) * (KT // 2)):
                nc.tensor.matmul(
                    psA[j],
                    lhsT=w_sb[j][:, k, :],
                    rhs=v_sb[0][:, k, :],
                    start=(k == 0),
                    stop=(k == KT - 1),
                )
            if kh == 1:
                drain(j, 0, psA[j])

    # --- Main phase: per j, k-mid / t-inner over t1..t3 ------------------
    for j in range(JT):
        ps = [
            pspool.tile([P, TS], _FP32, name=f"ps_{j}_{t}", tag="ps")
            for t in range(1, NT)
        ]
        if j < JT - 1:
            for k in range(KT):
                for ti, t in enumerate(range(1, NT)):
                    nc.tensor.matmul(
                        ps[ti],
                        lhsT=w_sb[j][:, k, :],
                        rhs=v_sb[t][:, k, :],
                        start=(k == 0),
                        stop=(k == KT - 1),
                    )
            for ti, t in enumerate(range(1, NT)):
                drain(j, t, ps[ti])
        else:
            # tail j: t-mid / k-inner so drains + per-t output DMAs overlap
            # the remaining matmuls; only the last 64KB DMA trails.
            nc.sync.dma_start(
                out=outT[j * P : (j + 1) * P, 0:TS], in_=ob_sb[j][:, 0:TS]
            )
            for ti, t in enumerate(range(1, NT)):
                if t < NT - 1:
                    for k in range(KT):
                        nc.tensor.matmul(
                            ps[ti],
                            lhsT=w_sb[j][:, k, :],
                            rhs=v_sb[t][:, k, :],
                            start=(k == 0),
                            stop=(k == KT - 1),
                        )
                    drain(j, t, ps[ti])
                    nc.sync.dma_start(
                        out=outT[j * P : (j + 1) * P, t * TS : (t + 1) * TS],
                        in_=ob_sb[j][:, t * TS : (t + 1) * TS],
                    )
                else:
                    # final segment in two sequential halves with separate
                    # PSUM tiles: the first half's drain+DMA overlap the
                    # second half's matmuls; only a 64KB DMA trails the
                    # last matmul.
                    for h in range(2):
                        lo = t * TS + h * (TS // 2)
                        ph_full = pspool.tile(
                            [P, TS], _FP32, name=f"ps_tail_{h}", tag="ps"
                        )
                        ph = ph_full[:, : TS // 2]
                        for k in range(KT):
                            nc.tensor.matmul(
                                ph,
                                lhsT=w_sb[j][:, k, :],
                                rhs=v_sb[t][:, k, h * (TS // 2) : (h + 1) * (TS // 2)],
                                start=(k == 0),
                                stop=(k == KT - 1),
                            )
                        nc.vector.tensor_scalar(
                            out=ob_sb[j][:, lo : lo + TS // 2],
                            in0=ph,
                            scalar1=b_sb[:, j : j + 1],
                            scalar2=None,
                            op0=mybir.AluOpType.add,
                        )
                        nc.sync.dma_start(
                            out=outT[j * P : (j + 1) * P, lo : lo + TS // 2],
                            in_=ob_sb[j][:, lo : lo + TS // 2],
                        )
            return
        nc.sync.dma_start(out=outT[j * P : (j + 1) * P, :], in_=ob_sb[j])


_nc_cache = None


def _get_nc():
    global _nc_cache
    if _nc_cache is None:
        _nc_cache = _build()
    return _nc_cache


def prepare_inputs(inputs):
    """Host-side prep shared by kernel() and the timing harness."""
    import ml_dtypes

    v = np.ascontiguousarray(np.asarray(inputs["v"], dtype=np.float32))
    Wv = np.asarray(inputs["Wv"], dtype=np.float32)
    W0 = np.asarray(inputs["W0"], dtype=np.float32)
    b0 = np.asarray(inputs["b0"], dtype=np.float32)

    # Fuse the two linear layers on the host: WcT = (W0 @ Wcat)^T
    Wc = W0 @ Wv.reshape(H * D, C)  # [C_out, C_in]
    wT = np.ascontiguousarray(Wc.T.astype(ml_dtypes.bfloat16))  # [C_in, C_out]
    # w chunks relaid to [P, (k, j-within-chunk)] for contiguous DMA.
    wA = wT.reshape(KT, P, C)
    wR0 = np.ascontiguousarray(wA[:, :, 0:P].transpose(1, 0, 2).reshape(P, -1))
    wR1 = np.ascontiguousarray(wA[:, :, P : 4 * P].transpose(1, 0, 2).reshape(P, -1))
    wR2 = np.ascontiguousarray(wA[:, :, 4 * P : C].transpose(1, 0, 2).reshape(P, -1))
    bT = np.ascontiguousarray(b0.reshape(JT, P).T.astype(np.float32))  # [P, JT]
    vT = np.ascontiguousarray(
        v.transpose(0, 2, 1).astype(ml_dtypes.bfloat16)
    )  # [B, C, T]
    # vR: all strips relaid to [P, (strip, k, t)] so every device DMA is
    # fully contiguous per partition.
    vR = np.ascontiguousarray(
        vT.reshape(B, KT, P, NT, TS)
        .transpose(0, 2, 3, 1, 4)
        .reshape(B, P, NT * KT * TS)
    )
    return [
        {"vR": vR[i], "wR0": wR0, "wR1": wR1, "wR2": wR2, "bT": bT}
        for i in range(B)
    ]


def kernel(**inputs):
    in_maps = prepare_inputs(inputs)
    nc = _get_nc()
    res = run_bass_kernel_spmd(nc, in_maps, core_ids=list(range(B)))
    return np.stack(
        [
            np.ascontiguousarray(res.results[i]["outT"].T).astype(np.float32)
            for i in range(B)
        ],
        axis=0,
    )
